# revision 1
# baseline (speedup 1.0000x reference)
"""Trainium2 Bass kernel for nn_Discriminator_54511724921068 (2x EdgeConv GNN).

8 NeuronCores: 2 batches x 4 row-shards of N=4096 -> 1024 rows/core.
Per core: kNN via PE s-matrix + DVE max8 top-40; EdgeConv via u/v decomposition
(neighbor gather with indirect DMA); GroupNorm stats via tiny AllGather
collectives; faithful dist-channel view-scramble via AllGather + row_ids gather.
Self-contained: builds, compiles and runs the SPMD program on cores 0-7.
"""
import os
import numpy as np

ABL = os.environ.get("KABL", "")
N = 4096
B = 2
SH = 4
R = N // SH       # 1024 rows per core
NT = R // 128     # 8 row tiles
KN = 20
KR = 40
EPS_GN = 1e-5
NEG = -3.0e38

_CACHE = {}


def _build():
    import concourse.bass as bass
    import concourse.bacc as bacc
    import concourse.mybir as mybir
    from concourse.tile import TileContext

    dt = mybir.dt
    AX = mybir.AxisListType.X
    AF = mybir.ActivationFunctionType
    ALU = mybir.AluOpType
    RG = [[0, 1, 2, 3], [4, 5, 6, 7]]

    nc = bacc.Bacc(num_devices=8)

    def din(name, shape, d=dt.float32):
        return nc.dram_tensor(name, shape, d, kind="ExternalInput")

    pos_full_d = din("pos_full", [3, N])
    x_full_d = din("x_full", [16, N])
    pos_own_d = din("pos_own", [3, R])
    x_own_d = din("x_own", [16, R])
    u1w_d = din("u1w", [20, 64])
    v1w_d = din("v1w", [19, 64])
    wd1_d = din("wd1", [1, 64])
    w1bt_d = din("w1bt", [64, 64])
    u2w_d = din("u2w", [67, 128])
    v2w_d = din("v2w", [67, 128])
    wd2_d = din("wd2", [1, 128])
    w2bt_d = din("w2bt", [128, 128])
    gnp_d = {nm: din(nm, [1, 64]) for nm in ("g1a", "be1a", "g1b", "be1b")}
    gnp_d.update({nm: din(nm, [1, 128]) for nm in ("g2a", "be2a", "g2b", "be2b")})
    ident_d = din("ident", [128, 128])
    one_d = din("one", [1, 1])
    pairmask_d = din("pairmask", [64, 32])
    quadmask_d = din("quadmask", [128, 32])
    row_ids_d = din("row_ids", [128, NT], dt.uint32)

    out_d = nc.dram_tensor("x2_out", [128, R], dt.float32, kind="ExternalOutput")

    u1t_d = nc.dram_tensor("u1t", [N, 64], dt.float32, kind="Internal")
    u2t_own_d = nc.dram_tensor("u2t_own", [R, 128], dt.float32, kind="Internal")
    u2t_full_d = nc.dram_tensor("u2t_full", [N, 128], dt.float32, kind="Internal")
    dag_in_d = nc.dram_tensor("dag_in", [KN, R], dt.float32, kind="Internal")
    dag_out_d = nc.dram_tensor("dag_out", [4, KN, R], dt.float32, kind="Internal")
    dist_rows_d = nc.dram_tensor("dist_rows", [N, KN], dt.float32, kind="Internal")
    y1t_d = nc.dram_tensor("y1t_spill", [NT, 64, KN * 128], dt.float32,
                           kind="Internal")
    y2t_d = nc.dram_tensor("y2t_spill", [NT, 128, KN * 128], dt.float32,
                           kind="Internal")
    stat_in_d = [nc.dram_tensor(f"stat_in{i}", [128, 2], dt.float32, kind="Internal")
                 for i in range(4)]
    stat_out_d = [nc.dram_tensor(f"stat_out{i}", [4, 128, 2], dt.float32,
                                 kind="Internal") for i in range(4)]

    with TileContext(nc) as tc:
        with tc.tile_pool(name="const", bufs=1) as cp, \
             tc.tile_pool(name="scratch1", bufs=2) as s1p, \
             tc.tile_pool(name="sb", bufs=3) as sp, \
             tc.tile_pool(name="big", bufs=2) as bp, \
             tc.tile_pool(name="persist", bufs=1) as pp, \
             tc.tile_pool(name="ps", bufs=3, space="PSUM") as ps, \
             tc.tile_pool(name="psbig", bufs=1, space="PSUM") as psb:

            MM = dict(tag="mm512")
            ST = dict(tag="stage")

            def cload(name, shape, d_):
                t_ = cp.tile(shape, dt.float32, tag=name)
                nc.sync.dma_start(out=t_, in_=d_[:])
                return t_
            ident = cload("ident", [128, 128], ident_d)
            one = cload("one", [1, 1], one_d)
            pairmask = cload("pairmask", [64, 32], pairmask_d)
            quadmask = cload("quadmask", [128, 32], quadmask_d)
            u1w = cload("u1w", [20, 64], u1w_d)
            v1w = cload("v1w", [19, 64], v1w_d)
            w1bt = cload("w1bt", [64, 64], w1bt_d)
            u2w = cload("u2w", [67, 128], u2w_d)
            v2w = cload("v2w", [67, 128], v2w_d)
            w2bt = cload("w2bt", [128, 128], w2bt_d)
            gn = {nm: cload(nm, [1, 64], gnp_d[nm])
                  for nm in ("g1a", "be1a", "g1b", "be1b")}
            gn.update({nm: cload(nm, [1, 128], gnp_d[nm])
                       for nm in ("g2a", "be2a", "g2b", "be2b")})
            wd1_row = cload("wd1", [1, 64], wd1_d)
            wd2_row = cload("wd2", [1, 128], wd2_d)
            row_ids = cp.tile([128, NT], dt.uint32, tag="row_ids")
            nc.sync.dma_start(out=row_ids, in_=row_ids_d[:])

            ones3 = cp.tile([3, 1], dt.float32, tag="ones3")
            nc.vector.memset(ones3, 1.0)
            ones128c = cp.tile([1, 128], dt.float32, tag="ones128c")
            nc.vector.memset(ones128c, 1.0)
            eps3_t = cp.tile([128, 1], dt.float32, tag="eps3")
            nc.vector.memset(eps3_t, 3.0e-12)
            epsgn_t = cp.tile([1, 1], dt.float32, tag="epsgn")
            nc.vector.memset(epsgn_t, EPS_GN)

            # ---------- P1: inputs; xp_full rows: pos(0-2), -sq(3), x(4-19) ----
            xp_full = pp.tile([20, N], dt.float32)
            nc.sync.dma_start(out=xp_full[0:3, :], in_=pos_full_d[:])
            nc.sync.dma_start(out=xp_full[4:20, :], in_=x_full_d[:])
            xp_own = pp.tile([19, R], dt.float32)
            nc.sync.dma_start(out=xp_own[0:3, :], in_=pos_own_d[:])
            nc.sync.dma_start(out=xp_own[3:19, :], in_=x_own_d[:])
            pos_own = xp_own[0:3, :]

            p2 = s1p.tile([3, N], dt.float32, **ST)
            nc.vector.tensor_mul(out=p2, in0=xp_full[0:3, :], in1=xp_full[0:3, :])
            negsq = s1p.tile([1, N], dt.float32, tag="negsq", bufs=1)
            for j in range(N // 512):
                sqp = ps.tile([1, 512], dt.float32, **MM)
                nc.tensor.matmul(out=sqp, lhsT=ones3, rhs=p2[:, 512*j:512*(j+1)],
                                 start=True, stop=True)
                nc.scalar.activation(out=negsq[:, 512*j:512*(j+1)], in_=sqp,
                                     func=AF.Copy, scale=-1.0)
            nc.sync.dma_start(out=xp_full[3:4, :], in_=negsq)
            aug = xp_full[0:4, :]

            lhs_all = pp.tile([4, R], dt.float32)
            nc.scalar.activation(out=lhs_all[0:3, :], in_=pos_own, func=AF.Copy,
                                 scale=2.0)
            ones_r = s1p.tile([1, R], dt.float32, tag="ones_r", bufs=1)
            nc.vector.memset(ones_r, 1.0)
            nc.sync.dma_start(out=lhs_all[3:4, :], in_=ones_r)

            # sq_own per-partition per tile
            sq_ownT = pp.tile([128, NT], dt.float32)
            p2o = s1p.tile([3, R], dt.float32, **ST)
            nc.vector.tensor_mul(out=p2o, in0=pos_own, in1=pos_own)
            own_sq = s1p.tile([1, R], dt.float32, **ST)
            for j in range(R // 512):
                sqp = ps.tile([1, 512], dt.float32, **MM)
                nc.tensor.matmul(out=sqp, lhsT=ones3, rhs=p2o[:, 512*j:512*(j+1)],
                                 start=True, stop=True)
                nc.scalar.activation(out=own_sq[:, 512*j:512*(j+1)], in_=sqp,
                                     func=AF.Copy)
            for t in range(NT):
                tp = ps.tile([128, 1], dt.float32, **MM)
                nc.tensor.matmul(out=tp, lhsT=own_sq[:, 128*t:128*(t+1)], rhs=one,
                                 start=True, stop=True)
                nc.scalar.activation(out=sq_ownT[:, t:t+1], in_=tp, func=AF.Copy)

            # ---------- P2: u1 -> u1T DRAM; v1 local ----------
            u1 = s1p.tile([64, N], dt.float32, **ST)
            for j in range(N // 512):
                up = ps.tile([64, 512], dt.float32, **MM)
                nc.tensor.matmul(out=up, lhsT=u1w, rhs=xp_full[0:20, 512*j:512*(j+1)],
                                 start=True, stop=True)
                nc.scalar.activation(out=u1[:, 512*j:512*(j+1)], in_=up, func=AF.Copy)
            u1t_sb = s1p.tile([128, 32 * 64], dt.float32, **ST)
            for j in range(32):
                tpp = ps.tile([128, 128], dt.float32, **MM)
                nc.tensor.transpose(out=tpp[:, 0:64], in_=u1[:, 128*j:128*(j+1)],
                                    identity=ident[0:64, 0:64])
                nc.scalar.activation(out=u1t_sb[:, 64*j:64*(j+1)], in_=tpp[:, 0:64],
                                     func=AF.Copy)
            nc.sync.dma_start(
                out=u1t_d[:].rearrange("(j p) c -> p j c", p=128),
                in_=u1t_sb.rearrange("p (j c) -> p j c", c=64))

            v1 = pp.tile([64, R], dt.float32)
            for j in range(R // 512):
                vp = ps.tile([64, 512], dt.float32, **MM)
                nc.tensor.matmul(out=vp, lhsT=v1w, rhs=xp_own[:, 512*j:512*(j+1)],
                                 start=True, stop=True)
                nc.scalar.activation(out=v1[:, 512*j:512*(j+1)], in_=vp, func=AF.Copy)

            # wd k-replicated broadcast tiles
            def wd_make(row, C, sfx):
                bc = sp.tile([128, C], dt.float32, tag=f"wdbc{sfx}")
                wdp = ps.tile([128, C], dt.float32, **MM)
                nc.tensor.matmul(out=wdp, lhsT=ones128c, rhs=row, start=True,
                                 stop=True)
                nc.scalar.activation(out=bc, in_=wdp, func=AF.Copy)
                kc = cp.tile([128, KN * C], dt.float32, tag=f"wdkc{sfx}")
                for k in range(KN):
                    nc.vector.tensor_copy(out=kc[:, C*k:C*(k+1)], in_=bc)
                return kc
            wd1_kc = wd_make(wd1_row, 64, "1")
            wd2_kc = wd_make(wd2_row, 128, "2")

            # ---------- loop A: s matmul, topk, dist ----------
            idx_t = []
            dist_ownT = pp.tile([KN, R], dt.float32)
            for t in range(NT):
                s_sb = s1p.tile([128, N], dt.float32, **ST)
                for j in range(N // 512):
                    sps = ps.tile([128, 512], dt.float32, **MM)
                    nc.tensor.matmul(out=sps, lhsT=lhs_all[:, 128*t:128*(t+1)],
                                     rhs=aug[:, 512*j:512*(j+1)], start=True,
                                     stop=True)
                    nc.scalar.activation(out=s_sb[:, 512*j:512*(j+1)], in_=sps,
                                         func=AF.Copy)
                vals = sp.tile([128, KR], dt.float32, tag="vals")
                idxs = pp.tile([128, KR], dt.uint32, tag=f"idxs{t}")
                nrounds = 1 if ABL == "notopk" else 5
                for r_ in range(nrounds):
                    v8 = vals[:, 8*r_:8*r_+8]
                    nc.vector.max(out=v8, in_=s_sb)
                    nc.vector.max_index(out=idxs[:, 8*r_:8*r_+8], in_max=v8,
                                        in_values=s_sb)
                    nc.vector.match_replace(out=s_sb, in_to_replace=v8,
                                            in_values=s_sb, imm_value=NEG)
                if ABL == "notopk":
                    nc.vector.memset(idxs[:, 8:], 0)
                    nc.vector.memset(vals[:, 8:], 0.0)
                idx_t.append(idxs)
                dd = sp.tile([128, KN], dt.float32, tag="dd")
                nc.vector.tensor_scalar(out=dd, in0=vals[:, 0:KR:2],
                                        scalar1=sq_ownT[:, t:t+1], scalar2=0.0,
                                        op0=ALU.subtract, op1=ALU.min)
                dist = sp.tile([128, KN], dt.float32, tag="dist")
                nc.scalar.activation(out=dist, in_=dd, func=AF.Sqrt, scale=-1.0,
                                     bias=eps3_t[:, 0:1])
                dtp = ps.tile([KN, 128], dt.float32, **MM)
                nc.tensor.transpose(out=dtp, in_=dist, identity=ident)
                nc.scalar.activation(out=dist_ownT[:, 128*t:128*(t+1)], in_=dtp,
                                     func=AF.Copy)

            # ---------- dist scramble: AllGather + rearrange ----------
            nc.sync.dma_start(out=dag_in_d[:], in_=dist_ownT)
            nc.gpsimd.collective_compute(
                kind="AllGather", op=ALU.bypass, replica_groups=RG,
                ins=[dag_in_d[:]], outs=[dag_out_d[:]])
            ag_sb = sp.tile([4 * KN, R], dt.float32, tag="ag_sb")
            nc.sync.dma_start(out=ag_sb,
                              in_=dag_out_d[:].rearrange("c k j -> (c k) j"))
            # dist_rows viewed flat is dist_orig.T row-major:
            # flat[k''*4096 + n'] = dist_orig[n', k''] = ag_sb[20c + k'', j]
            drflat = dist_rows_d[:].rearrange("n k -> (n k)")
            for kpp in range(KN):
                nc.sync.dma_start(
                    out=drflat[N*kpp:N*(kpp+1)].rearrange("(c j) -> c j", c=4),
                    in_=ag_sb[kpp::KN, :])

            # ---------- GN helpers ----------
            def gn_allreduce(acc2, C, icc):
                nc.sync.dma_start(out=stat_in_d[icc][0:C, :], in_=acc2)
                nc.gpsimd.collective_compute(
                    kind="AllGather", op=ALU.bypass, replica_groups=RG,
                    ins=[stat_in_d[icc][:]], outs=[stat_out_d[icc][:]])
                accg = sp.tile([C, 8], dt.float32, tag="accg")
                nc.sync.dma_start(
                    out=accg.rearrange("p (c s) -> p c s", s=2),
                    in_=stat_out_d[icc][:, 0:C, :].rearrange("c p s -> p c s"))
                accf = sp.tile([C, 2], dt.float32, tag="accf")
                nc.vector.reduce_sum(accf, accg.rearrange("p (c s) -> p s c", s=2),
                                     axis=AX)
                return accf

            def gn_scale_shift(accf, C, mask, gamma, beta, count, icc):
                G2 = 32
                rep = C // G2
                grow_p = ps.tile([1, 2 * G2], dt.float32, **MM)
                nc.tensor.matmul(out=grow_p[:, 0:G2], lhsT=accf[:, 0:1], rhs=mask,
                                 start=True, stop=True)
                nc.tensor.matmul(out=grow_p[:, G2:2*G2], lhsT=accf[:, 1:2], rhs=mask,
                                 start=True, stop=True)
                grow = sp.tile([1, 2 * G2], dt.float32, tag="grow")
                nc.scalar.activation(out=grow, in_=grow_p, func=AF.Copy,
                                     scale=1.0 / count)
                var = sp.tile([1, G2], dt.float32, tag="var")
                nc.vector.tensor_mul(out=var, in0=grow[:, 0:G2], in1=grow[:, 0:G2])
                nc.vector.tensor_sub(out=var, in0=grow[:, G2:2*G2], in1=var)
                sd = sp.tile([1, G2], dt.float32, tag="sd")
                nc.scalar.activation(out=sd, in_=var, func=AF.Sqrt,
                                     bias=epsgn_t[0:1, 0:1])
                inv = sp.tile([1, G2], dt.float32, tag="inv")
                nc.vector.reciprocal(out=inv, in_=sd)
                scale_r = sp.tile([1, C], dt.float32, tag="scale_r")
                nc.vector.tensor_mul(
                    out=scale_r.rearrange("a (g r) -> a g r", g=G2),
                    in0=gamma.rearrange("a (g r) -> a g r", g=G2),
                    in1=inv.to_broadcast([1, G2, rep]))
                shift_r = sp.tile([1, C], dt.float32, tag="shift_r")
                nc.vector.tensor_mul(
                    out=shift_r.rearrange("a (g r) -> a g r", g=G2),
                    in0=scale_r.rearrange("a (g r) -> a g r", g=G2),
                    in1=grow[:, 0:G2].to_broadcast([1, G2, rep]))
                nc.vector.tensor_sub(out=shift_r, in0=beta, in1=shift_r)
                ssp = ps.tile([C, 2], dt.float32, **MM)
                nc.tensor.matmul(out=ssp[:, 0:1], lhsT=scale_r, rhs=one,
                                 start=True, stop=True)
                nc.tensor.matmul(out=ssp[:, 1:2], lhsT=shift_r, rhs=one,
                                 start=True, stop=True)
                ss = cp.tile([C, 6], dt.float32, tag=f"ss{icc}")
                nc.scalar.activation(out=ss[:, 0:2], in_=ssp, func=AF.Copy)
                nc.scalar.activation(out=ss[:, 2:4], in_=ssp, func=AF.Copy, scale=0.8)
                nc.scalar.activation(out=ss[:, 4:6], in_=ssp, func=AF.Copy, scale=0.2)
                return ss

            def lrelu_full(out, in_, ss, tmp):
                # out = 0.8*Relu(z) + 0.2*z, z = in_*scale + shift
                nc.scalar.activation(out=out, in_=in_, func=AF.Relu,
                                     scale=ss[:, 2:3], bias=ss[:, 3:4])
                nc.vector.tensor_scalar(out=tmp, in0=in_, scalar1=ss[:, 4:5],
                                        scalar2=ss[:, 5:6], op0=ALU.mult,
                                        op1=ALU.add)
                nc.vector.tensor_add(out=out, in0=out, in1=tmp)

            # ======== layer 1 ========
            # loop B: gather u1, ypre = g + dist*wd -> transpose -> +v1 -> spill
            dsc_t = []
            acc = sp.tile([64, 2 * NT], dt.float32, tag="acc64")
            for t in range(NT):
                g1 = bp.tile([128, KN, 64], dt.float32, tag="g")
                if ABL == "nogather":
                    nc.vector.memset(g1, 0.0)
                else:
                    for k in range(KN):
                        nc.gpsimd.indirect_dma_start(
                            out=g1[:, k, :], out_offset=None, in_=u1t_d[:],
                            in_offset=bass.IndirectOffsetOnAxis(
                                ap=idx_t[t][:, 2*k:2*k+1], axis=0))
                dsc = pp.tile([128, KN], dt.float32, tag=f"dsc{t}")
                nc.gpsimd.indirect_dma_start(
                    out=dsc, out_offset=None, in_=dist_rows_d[:],
                    in_offset=bass.IndirectOffsetOnAxis(ap=row_ids[:, t:t+1], axis=0))
                dsc_t.append(dsc)
                ypre = bp.tile([128, KN * 64], dt.float32, tag="ypre")
                nc.vector.tensor_mul(
                    out=ypre.rearrange("p (k c) -> p k c", c=64),
                    in0=dsc.to_broadcast([128, KN, 64]),
                    in1=wd1_kc.rearrange("p (k c) -> p k c", c=64))
                nc.vector.tensor_add(out=ypre, in0=ypre,
                                     in1=g1.rearrange("p k c -> p (k c)"))
                ytp = psb.tile([128, KN * 128], dt.float32, tag="ytp")
                for k in range(KN):
                    nc.tensor.transpose(out=ytp[0:64, 128*k:128*(k+1)],
                                        in_=ypre[:, 64*k:64*(k+1)], identity=ident)
                y1T = bp.tile([64, KN * 128], dt.float32, tag="h")
                nc.vector.tensor_add(
                    out=y1T.rearrange("c (k r) -> c r k", k=KN),
                    in0=ytp[0:64, :].rearrange("c (k r) -> c r k", k=KN),
                    in1=v1[:, 128*t:128*(t+1)].to_broadcast([64, 128, KN]))
                nc.sync.dma_start(out=y1t_d[t], in_=y1T)
                nc.vector.reduce_sum(acc[:, 2*t:2*t+1], y1T, axis=AX)
                trash = bp.tile([64, KN * 128], dt.float32, tag="ypre")
                nc.scalar.activation(out=trash, in_=y1T, func=AF.Square,
                                     accum_out=acc[:, 2*t+1:2*t+2])
            acc2 = sp.tile([64, 2], dt.float32, tag="acc2")
            nc.vector.reduce_sum(acc2, acc.rearrange("c (t s) -> c s t", s=2), axis=AX)
            accf = gn_allreduce(acc2, 64, 0)
            ss1a = gn_scale_shift(accf, 64, pairmask, gn["g1a"], gn["be1a"],
                                  2.0 * N * KN, 0)

            # loop C pass1: conv1b stats
            acc = sp.tile([64, 2 * NT], dt.float32, tag="acc64")
            for t in range(NT):
                y1T = bp.tile([64, KN * 128], dt.float32, tag="g")
                nc.sync.dma_start(out=y1T, in_=y1t_d[t])
                h = bp.tile([64, KN * 128], dt.float32, tag="h")
                tmp = bp.tile([64, KN * 128], dt.float32, tag="g")
                lrelu_full(h, y1T, ss1a, tmp)
                yb = bp.tile([64, KN * 128], dt.float32, tag="ypre")
                for j in range(KN * 128 // 512):
                    cbp = ps.tile([64, 512], dt.float32, **MM)
                    nc.tensor.matmul(out=cbp, lhsT=w1bt, rhs=h[:, 512*j:512*(j+1)],
                                     start=True, stop=True)
                    nc.scalar.activation(out=yb[:, 512*j:512*(j+1)], in_=cbp,
                                         func=AF.Copy)
                nc.vector.reduce_sum(acc[:, 2*t:2*t+1], yb, axis=AX)
                trash = bp.tile([64, KN * 128], dt.float32, tag="h")
                nc.scalar.activation(out=trash, in_=yb, func=AF.Square,
                                     accum_out=acc[:, 2*t+1:2*t+2])
            acc2 = sp.tile([64, 2], dt.float32, tag="acc2")
            nc.vector.reduce_sum(acc2, acc.rearrange("c (t s) -> c s t", s=2), axis=AX)
            accf = gn_allreduce(acc2, 64, 1)
            ss1b = gn_scale_shift(accf, 64, pairmask, gn["g1b"], gn["be1b"],
                                  2.0 * N * KN, 1)

            # loop D: recompute conv1b, normalize-b, max over k -> x1
            x1 = pp.tile([64, R], dt.float32)
            for t in range(NT):
                y1T = bp.tile([64, KN * 128], dt.float32, tag="g")
                nc.sync.dma_start(out=y1T, in_=y1t_d[t])
                h = bp.tile([64, KN * 128], dt.float32, tag="h")
                tmp = bp.tile([64, KN * 128], dt.float32, tag="g")
                lrelu_full(h, y1T, ss1a, tmp)
                hb = bp.tile([64, KN * 128], dt.float32, tag="ypre")
                for j in range(KN * 128 // 512):
                    cbp = ps.tile([64, 512], dt.float32, **MM)
                    nc.tensor.matmul(out=cbp, lhsT=w1bt, rhs=h[:, 512*j:512*(j+1)],
                                     start=True, stop=True)
                    tch = sp.tile([64, 512], dt.float32, tag="tch", bufs=2)
                    lrelu_full(hb[:, 512*j:512*(j+1)], cbp, ss1b, tch)
                nc.vector.reduce_max(x1[:, 128*t:128*(t+1)],
                                     hb.rearrange("c (k r) -> c r k", k=KN), axis=AX)

            # ======== layer 2 ========
            xp2 = pp.tile([67, R], dt.float32)
            nc.vector.tensor_copy(out=xp2[0:64, :], in_=x1)
            nc.vector.tensor_copy(out=xp2[64:67, :], in_=pos_own)
            u2 = s1p.tile([128, R], dt.float32, **ST)
            v2 = pp.tile([128, R], dt.float32)
            for j in range(R // 512):
                up2 = ps.tile([128, 512], dt.float32, **MM)
                nc.tensor.matmul(out=up2, lhsT=u2w, rhs=xp2[:, 512*j:512*(j+1)],
                                 start=True, stop=True)
                nc.scalar.activation(out=u2[:, 512*j:512*(j+1)], in_=up2, func=AF.Copy)
                vp2 = ps.tile([128, 512], dt.float32, **MM)
                nc.tensor.matmul(out=vp2, lhsT=v2w, rhs=xp2[:, 512*j:512*(j+1)],
                                 start=True, stop=True)
                nc.scalar.activation(out=v2[:, 512*j:512*(j+1)], in_=vp2, func=AF.Copy)
            u2t_sb = s1p.tile([128, 8 * 128], dt.float32, **ST)
            for j in range(8):
                tp2 = ps.tile([128, 128], dt.float32, **MM)
                nc.tensor.transpose(out=tp2, in_=u2[:, 128*j:128*(j+1)],
                                    identity=ident)
                nc.scalar.activation(out=u2t_sb[:, 128*j:128*(j+1)], in_=tp2,
                                     func=AF.Copy)
            nc.sync.dma_start(
                out=u2t_own_d[:].rearrange("(j p) c -> p j c", p=128),
                in_=u2t_sb.rearrange("p (j c) -> p j c", c=128))
            nc.gpsimd.collective_compute(
                kind="AllGather", op=ALU.bypass, replica_groups=RG,
                ins=[u2t_own_d[:]], outs=[u2t_full_d[:]])

            # loop E: gathers, ypre2, transpose, +v2, spill + stats-a
            acc = sp.tile([128, 2 * NT], dt.float32, tag="acc128")
            for t in range(NT):
                g2 = bp.tile([128, KN, 128], dt.float32, tag="g")
                if ABL == "nogather":
                    nc.vector.memset(g2, 0.0)
                else:
                    for k in range(KN):
                        nc.gpsimd.indirect_dma_start(
                            out=g2[:, k, :], out_offset=None, in_=u2t_full_d[:],
                            in_offset=bass.IndirectOffsetOnAxis(
                                ap=idx_t[t][:, 2*k:2*k+1], axis=0))
                ypre = bp.tile([128, KN * 128], dt.float32, tag="ypre")
                nc.vector.tensor_mul(
                    out=ypre.rearrange("p (k c) -> p k c", c=128),
                    in0=dsc_t[t].to_broadcast([128, KN, 128]),
                    in1=wd2_kc.rearrange("p (k c) -> p k c", c=128))
                nc.vector.tensor_add(out=ypre, in0=ypre,
                                     in1=g2.rearrange("p k c -> p (k c)"))
                ytp = psb.tile([128, KN * 128], dt.float32, tag="ytp")
                for k in range(KN):
                    nc.tensor.transpose(out=ytp[:, 128*k:128*(k+1)],
                                        in_=ypre[:, 128*k:128*(k+1)], identity=ident)
                y2T = bp.tile([128, KN * 128], dt.float32, tag="h")
                nc.vector.tensor_add(
                    out=y2T.rearrange("c (k r) -> c r k", k=KN),
                    in0=ytp.rearrange("c (k r) -> c r k", k=KN),
                    in1=v2[:, 128*t:128*(t+1)].to_broadcast([128, 128, KN]))
                nc.sync.dma_start(out=y2t_d[t], in_=y2T)
                nc.vector.reduce_sum(acc[:, 2*t:2*t+1], y2T, axis=AX)
                trash = bp.tile([128, KN * 128], dt.float32, tag="ypre")
                nc.scalar.activation(out=trash, in_=y2T, func=AF.Square,
                                     accum_out=acc[:, 2*t+1:2*t+2])
            acc2 = sp.tile([128, 2], dt.float32, tag="acc2")
            nc.vector.reduce_sum(acc2, acc.rearrange("c (t s) -> c s t", s=2), axis=AX)
            accf = gn_allreduce(acc2, 128, 2)
            ss2a = gn_scale_shift(accf, 128, quadmask, gn["g2a"], gn["be2a"],
                                  4.0 * N * KN, 2)

            # loop F pass1: conv2b stats
            acc = sp.tile([128, 2 * NT], dt.float32, tag="acc128")
            for t in range(NT):
                y2T = bp.tile([128, KN * 128], dt.float32, tag="g")
                nc.sync.dma_start(out=y2T, in_=y2t_d[t])
                h = bp.tile([128, KN * 128], dt.float32, tag="h")
                tmp = bp.tile([128, KN * 128], dt.float32, tag="g")
                lrelu_full(h, y2T, ss2a, tmp)
                yb = bp.tile([128, KN * 128], dt.float32, tag="ypre")
                for j in range(KN * 128 // 512):
                    cbp = ps.tile([128, 512], dt.float32, **MM)
                    nc.tensor.matmul(out=cbp, lhsT=w2bt, rhs=h[:, 512*j:512*(j+1)],
                                     start=True, stop=True)
                    nc.scalar.activation(out=yb[:, 512*j:512*(j+1)], in_=cbp,
                                         func=AF.Copy)
                nc.vector.reduce_sum(acc[:, 2*t:2*t+1], yb, axis=AX)
                trash = bp.tile([128, KN * 128], dt.float32, tag="h")
                nc.scalar.activation(out=trash, in_=yb, func=AF.Square,
                                     accum_out=acc[:, 2*t+1:2*t+2])
            acc2 = sp.tile([128, 2], dt.float32, tag="acc2")
            nc.vector.reduce_sum(acc2, acc.rearrange("c (t s) -> c s t", s=2), axis=AX)
            accf = gn_allreduce(acc2, 128, 3)
            ss2b = gn_scale_shift(accf, 128, quadmask, gn["g2b"], gn["be2b"],
                                  4.0 * N * KN, 3)

            # loop G: recompute conv2b, normalize-b, max -> out
            for t in range(NT):
                y2T = bp.tile([128, KN * 128], dt.float32, tag="g")
                nc.sync.dma_start(out=y2T, in_=y2t_d[t])
                h = bp.tile([128, KN * 128], dt.float32, tag="h")
                tmp = bp.tile([128, KN * 128], dt.float32, tag="g")
                lrelu_full(h, y2T, ss2a, tmp)
                hb = bp.tile([128, KN * 128], dt.float32, tag="ypre")
                for j in range(KN * 128 // 512):
                    cbp = ps.tile([128, 512], dt.float32, **MM)
                    nc.tensor.matmul(out=cbp, lhsT=w2bt, rhs=h[:, 512*j:512*(j+1)],
                                     start=True, stop=True)
                    tch = sp.tile([128, 512], dt.float32, tag="tch", bufs=2)
                    lrelu_full(hb[:, 512*j:512*(j+1)], cbp, ss2b, tch)
                x2 = sp.tile([128, 128], dt.float32, tag="x2")
                nc.vector.reduce_max(x2, hb.rearrange("c (k r) -> c r k", k=KN),
                                     axis=AX)
                nc.sync.dma_start(out=out_d[:, 128*t:128*(t+1)], in_=x2)

    nc.compile()
    return nc


def _host_inputs(x, pos, w1a, b1a, g1a, be1a, w1b, b1b, g1b, be1b,
                 w2a, b2a, g2a, be2a, w2b, b2b, g2b, be2b):
    f32 = np.float32
    x = np.asarray(x); pos = np.asarray(pos)

    def prep(wa, C, pos_first):
        wa = np.asarray(wa)
        wn = wa[:, 0:C]; wp = wa[:, C:C+3]; wd = wa[:, C+3]; wc = wa[:, C+4:]
        if pos_first:
            # kernel layout rows: pos(3), -sq(1, zero weight), x(C)
            uw = np.concatenate(
                [wp.T, np.zeros((1, wn.shape[0]), f32), wn.T], axis=0)
            vw = np.concatenate([(-wp).T, (wc - wn).T], axis=0)
        else:
            uw = np.concatenate([wn, wp], axis=1).T
            vw = np.concatenate([wc - wn, -wp], axis=1).T
        return (np.ascontiguousarray(uw).astype(f32),
                np.ascontiguousarray(vw).astype(f32), wd.astype(f32))

    u1w, v1w, wd1 = prep(w1a, 16, True)
    u2w, v2w, wd2 = prep(w2a, 64, False)
    ident = np.eye(128, dtype=f32)
    one = np.ones((1, 1), f32)
    pairmask = (np.arange(64)[:, None] // 2 == np.arange(32)[None, :]).astype(f32)
    quadmask = (np.arange(128)[:, None] // 4 == np.arange(32)[None, :]).astype(f32)
    common = dict(
        u1w=u1w, v1w=v1w, wd1=wd1.reshape(1, 64),
        w1bt=np.ascontiguousarray(np.asarray(w1b).T).astype(f32),
        u2w=u2w, v2w=v2w, wd2=wd2.reshape(1, 128),
        w2bt=np.ascontiguousarray(np.asarray(w2b).T).astype(f32),
        g1a=np.asarray(g1a, f32).reshape(1, 64),
        be1a=np.asarray(be1a, f32).reshape(1, 64),
        g1b=np.asarray(g1b, f32).reshape(1, 64),
        be1b=np.asarray(be1b, f32).reshape(1, 64),
        g2a=np.asarray(g2a, f32).reshape(1, 128),
        be2a=np.asarray(be2a, f32).reshape(1, 128),
        g2b=np.asarray(g2b, f32).reshape(1, 128),
        be2b=np.asarray(be2b, f32).reshape(1, 128),
        ident=ident, one=one, pairmask=pairmask, quadmask=quadmask,
    )
    maps = []
    for c in range(8):
        b, s = c // SH, c % SH
        m = dict(common)
        rid = (R * s + np.arange(128)[:, None]
               + 128 * np.arange(NT)[None, :]).astype(np.uint32)
        m.update(
            pos_full=np.ascontiguousarray(pos[b], f32),
            x_full=np.ascontiguousarray(x[b], f32),
            pos_own=np.ascontiguousarray(pos[b][:, R*s:R*(s+1)], f32),
            x_own=np.ascontiguousarray(x[b][:, R*s:R*(s+1)], f32),
            row_ids=rid,
        )
        maps.append(m)
    return maps


def kernel(**inputs):
    from concourse.bass_utils import run_bass_kernel_spmd
    if "nc" not in _CACHE:
        _CACHE["nc"] = _build()
    nc = _CACHE["nc"]
    in_maps = _host_inputs(**inputs)
    res = run_bass_kernel_spmd(nc, in_maps, core_ids=list(range(8)))
    out = np.zeros((B, 128, N), np.float32)
    for c in range(8):
        b, s = c // SH, c % SH
        out[b, :, R*s:R*(s+1)] = res.results[c]["x2_out"]
    return out

